# revision 43
# baseline (speedup 1.0000x reference)
"""Multi-head self-attention Trainium2 Bass kernel (8 NeuronCores).

Sharding: tensor-parallel over heads. Each core c owns heads {2c, 2c+1}
for all 4 batches:
  - projects q, k, v for its 2 heads (from the full x),
  - computes attention (softmax without max-subtraction: scores are O(+-20),
    safe in fp32),
  - computes the partial output projection with its 128-row slice of W_o.
The 8 partial outputs are summed on the host (the "all-reduce").

Layout strategy (everything transposed so no on-device transposes needed):
  xT   [e, l]    host-prepped
  qT2/kT2 [128=2h*64d, l]  from lhsT=W[e,d2] (2 heads packed), rhs=xT
  v2   [l, 128=2h*64d]     from lhsT=xT, rhs=[Wv_h0|Wv_h1]
  scoresT [lk, lq] = kT.T @ qT  (two heads as concurrent PE row-groups)
  expT = exp(scoresT) on ACT (bf16 out)
  avT' [128=2h*64d, lq]: col-tiled matmuls lhsT=v2-slice, rhs=expT
  denom [1, lq] per head: ones-vector matmuls (col strips 0 / 32)
  concatT = avT * (1/denom broadcast)  -> exactly the lhsT the out-proj needs
  out_partial[l, o] = concatT.T @ W_o_slice
"""

import os
import sys

import numpy as np
import ml_dtypes

import concourse.bass as bass
import concourse.tile as tile
from concourse import bacc, mybir
from concourse.bass_utils import run_bass_kernel_spmd

BF16 = mybir.dt.bfloat16
F32 = mybir.dt.float32
AF = mybir.ActivationFunctionType

B = 4
L = 2048
E = 1024
H = 16
D = 64
NCORES = 8
ET = E // 128  # 8 e-tiles
LT = L // 128  # 16 l-tiles
LQB = 512  # moving free-dim block
NLQ = L // LQB  # 4


def build_kernel(dbg=False):
    nc = bacc.Bacc("TRN2", target_bir_lowering=False, debug=False, num_devices=NCORES)

    xt_d = nc.dram_tensor("xt", [B, 128, ET, L], BF16, kind="ExternalInput")
    wq_d = nc.dram_tensor("wq", [128, ET, 128], BF16, kind="ExternalInput")
    wk_d = nc.dram_tensor("wk", [128, ET, 128], BF16, kind="ExternalInput")
    wv_d = nc.dram_tensor("wv", [128, ET, 128], BF16, kind="ExternalInput")
    wo_d = nc.dram_tensor("wo", [128, E], BF16, kind="ExternalInput")
    out_d = nc.dram_tensor("out", [B, L, E], F32, kind="ExternalOutput")
    if dbg:
        qt2_d = nc.dram_tensor("qt2_dbg", [128, B, L], BF16, kind="ExternalOutput")
        kt2_d = nc.dram_tensor("kt2_dbg", [128, B, L], BF16, kind="ExternalOutput")
        v2_d = nc.dram_tensor("v2_dbg", [128, B, LT, 130], BF16, kind="ExternalOutput")
        rcp_d = nc.dram_tensor("rcp_dbg", [B, NLQ, 2, 1, LQB], F32, kind="ExternalOutput")
        cc_d = nc.dram_tensor("cc_dbg", [B, 2, 64, L], BF16, kind="ExternalOutput")

    with tile.TileContext(nc) as tc:
        with (
            tc.tile_pool(name="persist", bufs=1) as pp,
            tc.tile_pool(name="xin", bufs=3) as xpool,
            tc.tile_pool(name="exp", bufs=3) as epool,
            tc.tile_pool(name="small", bufs=2) as spool,
            tc.tile_pool(name="outp", bufs=3) as opool,
            tc.tile_pool(name="concat", bufs=2) as cpool,
        ):
            # --- persistent SBUF residents ---
            wq_sb = pp.tile([128, ET, 128], BF16, tag="wq")
            wk_sb = pp.tile([128, ET, 128], BF16, tag="wk")
            wv_sb = pp.tile([128, ET, 128], BF16, tag="wv")
            wo_sb = pp.tile([128, E], BF16, tag="wo")
            qt2 = []
            kt2 = []
            v2 = []
            for b in range(B):
                qt2_b = pp.tile([128, L], BF16, tag=f"qt2_{b}")
                kt2_b = pp.tile([128, L], BF16, tag=f"kt2_{b}")
                v2_b = pp.tile([128, LT, 130], BF16, tag=f"v2_{b}")
                qt2.append(qt2_b)
                kt2.append(kt2_b)
                v2.append(v2_b)

            nc.sync.dma_start(wq_sb[:], wq_d[:])
            nc.sync.dma_start(wk_sb[:], wk_d[:])
            nc.sync.dma_start(wv_sb[:], wv_d[:])
            nc.sync.dma_start(wo_sb[:], wo_d[:])
            for b in range(B):
                nc.vector.memset(v2[b][:], 1.0)

            with tc.tile_pool(name="ps", bufs=1, space="PSUM") as ps:

                def proj_chunk_units(b, lc):
                    """Units (callables) projecting q/k/v for l-chunk lc of b."""
                    lsl = bass.ts(lc, LQB)
                    state = {}

                    def u_q():
                        xtile = xpool.tile([128, ET, LQB], BF16, tag="x")
                        nc.sync.dma_start(xtile[:], xt_d[b, :, :, lsl])
                        state["x"] = xtile
                        ps_q = ps.tile([128, LQB], F32, tag="proj", bufs=2)
                        for et in range(ET):
                            nc.tensor.matmul(
                                ps_q[:],
                                wq_sb[:, et, :],
                                xtile[:, et, :],
                                start=(et == 0),
                                stop=(et == ET - 1),
                            )
                        nc.vector.tensor_copy(qt2[b][:, lsl], ps_q[:])

                    def u_k():
                        xtile = state["x"]
                        ps_k = ps.tile([128, LQB], F32, tag="proj", bufs=2)
                        for et in range(ET):
                            nc.tensor.matmul(
                                ps_k[:],
                                wk_sb[:, et, :],
                                xtile[:, et, :],
                                start=(et == 0),
                                stop=(et == ET - 1),
                            )
                        nc.vector.tensor_copy(kt2[b][:, lsl], ps_k[:])

                    def u_v(j):
                        def f():
                            xtile = state["x"]
                            lt = lc * (LQB // 128) + j
                            ps_v = ps.tile([128, 128], F32, tag="proj", bufs=2)
                            for et in range(ET):
                                nc.tensor.matmul(
                                    ps_v[:],
                                    xtile[:, et, bass.ts(j, 128)],
                                    wv_sb[:, et, :],
                                    start=(et == 0),
                                    stop=(et == ET - 1),
                                )
                            nc.vector.tensor_copy(v2[b][:, lt, 0:64], ps_v[:, 0:64])
                            nc.vector.tensor_copy(
                                v2[b][:, lt, 65:129], ps_v[:, 64:128]
                            )

                        return f

                    return [u_q, u_k] + [u_v(j) for j in range(LQB // 128)]

                def attention_block(b, lq, cc, bg):
                    """Attention for (b, lq-block): heads as concurrent row-groups.

                    `bg` is an iterator of background-work callables; one is
                    emitted per lkt iteration to fill PE slack without long
                    chains starving the exp pipeline."""
                    lqsl = bass.ts(lq, LQB)
                    ps_av0 = ps.tile([65, LQB], F32, tag="av0", bufs=1)
                    ps_av1 = ps.tile([65, LQB], F32, tag="av1", bufs=1)
                    ps_av = [ps_av0, ps_av1]
                    e2_first = None
                    for lkt in range(LT):
                        ps_s = ps.tile([128, 2, LQB], F32, tag="s", bufs=2)
                        for h in range(2):
                            hsl = slice(h * 64, (h + 1) * 64)
                            nc.tensor.matmul(
                                ps_s[:, h, :],
                                kt2[b][hsl, bass.ts(lkt, 128)],
                                qt2[b][hsl, lqsl],
                                start=True,
                                stop=True,
                            )
                        e2 = epool.tile(
                            [128, 2, LQB], BF16, tag="e0" if lkt == 0 else "e"
                        )
                        nc.scalar.activation(e2[:], ps_s[:], AF.Exp)
                        if lkt == 0:
                            # lkt 0's av matmuls are emitted LAST (after lkt 15)
                            # so the chain-closing matmul consumes an exp tile
                            # finished long ago -- no ACT wait at block end.
                            e2_first = e2
                            continue
                        for h in range(2):
                            nc.tensor.matmul(
                                ps_av[h][:],
                                v2[b][:, lkt, h * 65 : (h + 1) * 65],
                                e2[:, h, :],
                                start=(lkt == 1),
                                stop=False,
                            )
                        u = next(bg, None)
                        if u is not None:
                            u()
                    for h in range(2):
                        nc.tensor.matmul(
                            ps_av[h][:],
                            v2[b][:, 0, h * 65 : (h + 1) * 65],
                            e2_first[:, h, :],
                            start=False,
                            stop=True,
                        )
                    for h in range(2):
                        av_sb = spool.tile([65, LQB], F32, tag=f"av_sb{h}")
                        if h == 0:
                            nc.scalar.copy(av_sb[:], ps_av[h][:])
                        else:
                            nc.vector.tensor_copy(av_sb[:], ps_av[h][:])
                        dn0 = spool.tile([1, LQB], F32, tag=f"dn0{h}")
                        nc.sync.dma_start(dn0[0:1, :], av_sb[64:65, :])
                        rbd = spool.tile([64, LQB], F32, tag=f"rbd{h}")
                        nc.gpsimd.partition_broadcast(rbd[:], dn0[0:1, :])
                        rbr = spool.tile([64, LQB], F32, tag=f"rbr{h}")
                        nc.vector.reciprocal_approx_fast(out=rbr[:], in_=rbd[:])
                        if h == 0:
                            nc.vector.tensor_mul(
                                cc[:64, lqsl], av_sb[0:64, :], rbr[:]
                            )
                        else:
                            cctmp = spool.tile([64, LQB], BF16, tag="cctmp")
                            nc.vector.tensor_mul(cctmp[:], av_sb[0:64, :], rbr[:])
                            nc.sync.dma_start(cc[64:128, lqsl], cctmp[:])
                        if dbg:
                            nc.sync.dma_start(rcp_d[b, lq, h], rbr[0:1, :])

                def outproj_units(b, cc):
                    def u(lt, oc):
                        def f():
                            ps_o = ps.tile([128, 512], F32, tag="proj", bufs=2)
                            nc.tensor.matmul(
                                ps_o[:],
                                cc[:, bass.ts(lt, 128)],
                                wo_sb[:, bass.ts(oc, 512)],
                                start=True,
                                stop=True,
                            )
                            out_t = opool.tile([128, 512], F32, tag="out")
                            nc.vector.tensor_copy(out_t[:], ps_o[:])
                            nc.sync.dma_start(
                                out_d[b, bass.ts(lt, 128), bass.ts(oc, 512)], out_t[:]
                            )

                        return f

                    return [u(lt, oc) for lt in range(LT) for oc in range(2)]

                # prologue: project batch 0; under attention(b, lq) spread
                # proj(b+1) units and outproj(b, lq-1) units (those need only
                # the cc columns written by the previous lq block).
                for lc in range(NLQ):
                    for u in proj_chunk_units(0, lc):
                        u()
                for b in range(B):
                    cc = cpool.tile([128, L], BF16, tag="cc")
                    op_units = outproj_units(b, cc)  # 8 per lq block
                    for lq in range(NLQ):
                        units = []
                        if b + 1 < B:
                            units += proj_chunk_units(b + 1, lq)
                        if lq > 0:
                            units += op_units[8 * (lq - 1) : 8 * lq]
                        bg = iter(units)
                        attention_block(b, lq, cc, bg)
                        for u in bg:
                            u()
                    for u in op_units[8 * (NLQ - 1) :]:
                        u()
                    if dbg:
                        nc.sync.dma_start(cc_d[b, 0], cc[0:64, :])
                        nc.sync.dma_start(cc_d[b, 1], cc[64:128, :])

            if dbg:
                for b in range(B):
                    nc.sync.dma_start(qt2_d[:, b, :], qt2[b][:])
                    nc.sync.dma_start(kt2_d[:, b, :], kt2[b][:])
                    nc.sync.dma_start(v2_d[:, b], v2[b][:])

    nc.compile()
    return nc


def prep_inputs(x, W_q, W_k, W_v, W_o):
    """Build the 8 per-core input maps (numpy, host-side)."""
    bf = ml_dtypes.bfloat16
    # xT: [b, e, l] -> [b, ep(128), et(8), l]
    xt = np.ascontiguousarray(x.transpose(0, 2, 1)).reshape(B, ET, 128, L)
    xt = np.ascontiguousarray(xt.transpose(0, 2, 1, 3)).astype(bf)

    in_maps = []
    for c in range(NCORES):
        h0, h1 = 2 * c, 2 * c + 1
        # [e, 2*64] -> [ep, et, 128]
        def pack(w, scale=1.0):
            m = np.concatenate([w[h0] * scale, w[h1] * scale], axis=1)  # [E, 128]
            m = m.reshape(ET, 128, 128).transpose(1, 0, 2)  # [ep, et, 128]
            return np.ascontiguousarray(m).astype(bf)

        in_maps.append(
            {
                "xt": xt,
                "wq": pack(W_q, 0.125),
                "wk": pack(W_k),
                "wv": pack(W_v),
                "wo": np.ascontiguousarray(W_o[128 * c : 128 * (c + 1), :]).astype(bf),
            }
        )
    return in_maps


def _ensure_ntff_hook():
    """Register the axon NTFF profile hook if the image's antenv lacks it."""
    import types

    try:
        from antenv.axon_hooks import get_axon_ntff_profile_hook  # noqa: F401

        return
    except ImportError:
        pass
    try:
        from trn_agent_boot.trn_boot import _ntff_profile_via_ctypes
    except ImportError:
        return
    so = "/opt/axon/libaxon_pjrt.so"
    if not os.path.exists(so):
        return
    hook = _ntff_profile_via_ctypes(so)
    mod = types.ModuleType("antenv.axon_hooks")
    state = {"hook": hook}
    mod.get_axon_ntff_profile_hook = lambda: state["hook"]
    mod.set_axon_ntff_profile_hook = lambda h: state.update(hook=h)
    import antenv

    antenv.axon_hooks = mod
    sys.modules["antenv.axon_hooks"] = mod


_NC_CACHE = {}


def kernel(x, W_q, W_k, W_v, W_o):
    x = np.asarray(x, dtype=np.float32)
    W_q = np.asarray(W_q, dtype=np.float32)
    W_k = np.asarray(W_k, dtype=np.float32)
    W_v = np.asarray(W_v, dtype=np.float32)
    W_o = np.asarray(W_o, dtype=np.float32)

    if "nc" not in _NC_CACHE:
        _NC_CACHE["nc"] = build_kernel()
    nc = _NC_CACHE["nc"]

    in_maps = prep_inputs(x, W_q, W_k, W_v, W_o)
    if bool(int(os.environ.get("KERNEL_TRACE", "0"))):
        _ensure_ntff_hook()
    res = run_bass_kernel_spmd(
        nc,
        in_maps,
        core_ids=list(range(NCORES)),
        trace=bool(int(os.environ.get("KERNEL_TRACE", "0"))),
    )
    _NC_CACHE["last_results"] = res
    out = np.zeros((B, L, E), dtype=np.float32)
    for r in res.results:
        out += r["out"]
    return out


if __name__ == "__main__":
    # smoke test with random data
    rng = np.random.default_rng(0)
    x = rng.standard_normal((B, L, E), dtype=np.float32)
    wq = (rng.standard_normal((H, E, D)) / np.sqrt(E)).astype(np.float32)
    wk = (rng.standard_normal((H, E, D)) / np.sqrt(E)).astype(np.float32)
    wv = (rng.standard_normal((H, E, D)) / np.sqrt(E)).astype(np.float32)
    wo = (rng.standard_normal((E, E)) / np.sqrt(E)).astype(np.float32)
    out = kernel(x, wq, wk, wv, wo)
    print("out", out.shape, out.dtype, np.abs(out).max())


# revision 45
# speedup vs baseline: 1.0795x; 1.0795x over previous
"""Multi-head self-attention Trainium2 Bass kernel (8 NeuronCores).

Sharding: tensor-parallel over heads. Each core c owns heads {2c, 2c+1}
for all 4 batches:
  - projects q, k, v for its 2 heads (from the full x),
  - computes attention (softmax without max-subtraction: scores are O(+-20),
    safe in fp32),
  - computes the partial output projection with its 128-row slice of W_o.
The 8 partial outputs are summed on the host (the "all-reduce").

Layout strategy (everything transposed so no on-device transposes needed):
  xT   [e, l]    host-prepped
  qT2/kT2 [128=2h*64d, l]  from lhsT=W[e,d2] (2 heads packed), rhs=xT
  v2   [l, 128=2h*64d]     from lhsT=xT, rhs=[Wv_h0|Wv_h1]
  scoresT [lk, lq] = kT.T @ qT  (two heads as concurrent PE row-groups)
  expT = exp(scoresT) on ACT (bf16 out)
  avT' [128=2h*64d, lq]: col-tiled matmuls lhsT=v2-slice, rhs=expT
  denom [1, lq] per head: ones-vector matmuls (col strips 0 / 32)
  concatT = avT * (1/denom broadcast)  -> exactly the lhsT the out-proj needs
  out_partial[l, o] = concatT.T @ W_o_slice
"""

import os
import sys

import numpy as np
import ml_dtypes

import concourse.bass as bass
import concourse.tile as tile
from concourse import bacc, mybir
from concourse.bass_utils import run_bass_kernel_spmd

BF16 = mybir.dt.bfloat16
F32 = mybir.dt.float32
AF = mybir.ActivationFunctionType

B = 4
L = 2048
E = 1024
H = 16
D = 64
NCORES = 8
ET = E // 128  # 8 e-tiles
LT = L // 128  # 16 l-tiles
LQB = 512  # moving free-dim block
NLQ = L // LQB  # 4


def build_kernel(dbg=False):
    nc = bacc.Bacc("TRN2", target_bir_lowering=False, debug=False, num_devices=NCORES)

    xt_d = nc.dram_tensor("xt", [B, 128, ET, L], BF16, kind="ExternalInput")
    wq_d = nc.dram_tensor("wq", [128, ET, 128], BF16, kind="ExternalInput")
    wk_d = nc.dram_tensor("wk", [128, ET, 128], BF16, kind="ExternalInput")
    wv_d = nc.dram_tensor("wv", [128, ET, 128], BF16, kind="ExternalInput")
    wo_d = nc.dram_tensor("wo", [128, E], BF16, kind="ExternalInput")
    out_d = nc.dram_tensor("out", [B, L, E], F32, kind="ExternalOutput")
    if dbg:
        qt2_d = nc.dram_tensor("qt2_dbg", [128, B, L], BF16, kind="ExternalOutput")
        kt2_d = nc.dram_tensor("kt2_dbg", [128, B, L], BF16, kind="ExternalOutput")
        v2_d = nc.dram_tensor("v2_dbg", [128, B, LT, 130], BF16, kind="ExternalOutput")
        rcp_d = nc.dram_tensor("rcp_dbg", [B, NLQ, 2, 1, LQB], F32, kind="ExternalOutput")
        cc_d = nc.dram_tensor("cc_dbg", [B, 2, 64, L], BF16, kind="ExternalOutput")

    with tile.TileContext(nc) as tc:
        with (
            tc.tile_pool(name="persist", bufs=1) as pp,
            tc.tile_pool(name="xin", bufs=3) as xpool,
            tc.tile_pool(name="exp", bufs=3) as epool,
            tc.tile_pool(name="small", bufs=2) as spool,
            tc.tile_pool(name="outp", bufs=3) as opool,
            tc.tile_pool(name="concat", bufs=2) as cpool,
        ):
            # --- persistent SBUF residents ---
            wq_sb = pp.tile([128, ET, 128], BF16, tag="wq")
            wk_sb = pp.tile([128, ET, 128], BF16, tag="wk")
            wv_sb = pp.tile([128, ET, 128], BF16, tag="wv")
            wo_sb = pp.tile([128, E], BF16, tag="wo")
            qt2 = []
            kt2 = []
            v2 = []
            for b in range(B):
                qt2_b = pp.tile([128, L], BF16, tag=f"qt2_{b}")
                kt2_b = pp.tile([128, L], BF16, tag=f"kt2_{b}")
                v2_b = pp.tile([128, LT, 130], BF16, tag=f"v2_{b}")
                qt2.append(qt2_b)
                kt2.append(kt2_b)
                v2.append(v2_b)

            nc.sync.dma_start(wq_sb[:], wq_d[:])
            nc.sync.dma_start(wk_sb[:], wk_d[:])
            nc.sync.dma_start(wv_sb[:], wv_d[:])
            nc.sync.dma_start(wo_sb[:], wo_d[:])
            for b in range(B):
                nc.vector.memset(v2[b][:], 1.0)

            with tc.tile_pool(name="ps", bufs=1, space="PSUM") as ps:

                def proj_chunk_units(b, lc):
                    """Units (callables) projecting q/k/v for l-chunk lc of b."""
                    lsl = bass.ts(lc, LQB)
                    state = {}

                    def u_q():
                        xtile = xpool.tile([128, ET, LQB], BF16, tag="x")
                        nc.sync.dma_start(xtile[:], xt_d[b, :, :, lsl])
                        state["x"] = xtile
                        ps_q = ps.tile([128, LQB], F32, tag="proj", bufs=2)
                        for et in range(ET):
                            nc.tensor.matmul(
                                ps_q[:],
                                wq_sb[:, et, :],
                                xtile[:, et, :],
                                start=(et == 0),
                                stop=(et == ET - 1),
                            )
                        nc.vector.tensor_copy(qt2[b][:, lsl], ps_q[:])

                    def u_k():
                        xtile = state["x"]
                        ps_k = ps.tile([128, LQB], F32, tag="proj", bufs=2)
                        for et in range(ET):
                            nc.tensor.matmul(
                                ps_k[:],
                                wk_sb[:, et, :],
                                xtile[:, et, :],
                                start=(et == 0),
                                stop=(et == ET - 1),
                            )
                        nc.vector.tensor_copy(kt2[b][:, lsl], ps_k[:])

                    def u_v(j):
                        def f():
                            xtile = state["x"]
                            lt = lc * (LQB // 128) + j
                            ps_v = ps.tile([128, 128], F32, tag="proj", bufs=2)
                            for et in range(ET):
                                nc.tensor.matmul(
                                    ps_v[:],
                                    xtile[:, et, bass.ts(j, 128)],
                                    wv_sb[:, et, :],
                                    start=(et == 0),
                                    stop=(et == ET - 1),
                                )
                            nc.vector.tensor_copy(v2[b][:, lt, 0:64], ps_v[:, 0:64])
                            nc.vector.tensor_copy(
                                v2[b][:, lt, 65:129], ps_v[:, 64:128]
                            )

                        return f

                    return [u_q, u_k] + [u_v(j) for j in range(LQB // 128)]

                def attention_block(b, lq, cc, bg):
                    """Attention for (b, lq-block): heads as concurrent row-groups.

                    `bg` is an iterator of background-work callables; one is
                    emitted per lkt iteration to fill PE slack without long
                    chains starving the exp pipeline."""
                    lqsl = bass.ts(lq, LQB)
                    ps_av0 = ps.tile([65, LQB], F32, tag="av0", bufs=1)
                    ps_av1 = ps.tile([65, LQB], F32, tag="av1", bufs=1)
                    ps_av = [ps_av0, ps_av1]
                    e2_first = None
                    for lkt in range(LT):
                        ps_s = ps.tile([128, 2, LQB], F32, tag="s", bufs=2)
                        for h in range(2):
                            hsl = slice(h * 64, (h + 1) * 64)
                            nc.tensor.matmul(
                                ps_s[:, h, :],
                                kt2[b][hsl, bass.ts(lkt, 128)],
                                qt2[b][hsl, lqsl],
                                start=True,
                                stop=True,
                            )
                        e2 = epool.tile(
                            [128, 2, LQB], BF16, tag="e0" if lkt == 0 else "e"
                        )
                        nc.scalar.activation(e2[:], ps_s[:], AF.Exp)
                        if lkt == 0:
                            # lkt 0's av matmuls are emitted LAST (after lkt 15)
                            # so the chain-closing matmul consumes an exp tile
                            # finished long ago -- no ACT wait at block end.
                            e2_first = e2
                            continue
                        for h in range(2):
                            nc.tensor.matmul(
                                ps_av[h][:],
                                v2[b][:, lkt, h * 65 : (h + 1) * 65],
                                e2[:, h, :],
                                start=(lkt == 1),
                                stop=False,
                            )
                        u = next(bg, None)
                        if u is not None:
                            u()
                    for h in range(2):
                        nc.tensor.matmul(
                            ps_av[h][:],
                            v2[b][:, 0, h * 65 : (h + 1) * 65],
                            e2_first[:, h, :],
                            start=False,
                            stop=True,
                        )
                    for h in range(2):
                        av_sb = spool.tile([65, LQB], F32, tag=f"av_sb{h}")
                        nc.vector.tensor_copy(av_sb[:], ps_av[h][:])
                        dn0 = spool.tile([1, LQB], F32, tag=f"dn0{h}")
                        nc.sync.dma_start(dn0[0:1, :], av_sb[64:65, :])
                        rbd = spool.tile([64, LQB], F32, tag=f"rbd{h}")
                        nc.gpsimd.partition_broadcast(rbd[:], dn0[0:1, :])
                        rbr = spool.tile([64, LQB], F32, tag=f"rbr{h}")
                        nc.vector.reciprocal_approx_fast(out=rbr[:], in_=rbd[:])
                        if h == 0:
                            nc.vector.tensor_mul(
                                cc[:64, lqsl], av_sb[0:64, :], rbr[:]
                            )
                        else:
                            cctmp = spool.tile([64, LQB], BF16, tag="cctmp")
                            nc.vector.tensor_mul(cctmp[:], av_sb[0:64, :], rbr[:])
                            nc.sync.dma_start(cc[64:128, lqsl], cctmp[:])
                        if dbg:
                            nc.sync.dma_start(rcp_d[b, lq, h], rbr[0:1, :])

                def outproj_units(b, cc):
                    def u(lt, oc):
                        def f():
                            ps_o = ps.tile([128, 512], F32, tag="proj", bufs=2)
                            nc.tensor.matmul(
                                ps_o[:],
                                cc[:, bass.ts(lt, 128)],
                                wo_sb[:, bass.ts(oc, 512)],
                                start=True,
                                stop=True,
                            )
                            out_t = opool.tile([128, 512], F32, tag="out")
                            nc.vector.tensor_copy(out_t[:], ps_o[:])
                            nc.sync.dma_start(
                                out_d[b, bass.ts(lt, 128), bass.ts(oc, 512)], out_t[:]
                            )

                        return f

                    return [u(lt, oc) for lt in range(LT) for oc in range(2)]

                # prologue: project batch 0; spread proj(b+1) and
                # outproj(b-1) units under attention(b)'s lkt loop
                for lc in range(NLQ):
                    for u in proj_chunk_units(0, lc):
                        u()
                prev = None  # (b, cc) awaiting out-projection
                for b in range(B):
                    cc = cpool.tile([128, L], BF16, tag="cc")
                    units = []
                    if b + 1 < B:
                        for lc in range(NLQ):
                            units += proj_chunk_units(b + 1, lc)
                    if prev is not None:
                        units += outproj_units(prev[0], prev[1])
                    bg = iter(units)
                    for lq in range(NLQ):
                        attention_block(b, lq, cc, bg)
                    for u in bg:
                        u()
                    if dbg:
                        nc.sync.dma_start(cc_d[b, 0], cc[0:64, :])
                        nc.sync.dma_start(cc_d[b, 1], cc[64:128, :])
                    prev = (b, cc)
                for u in outproj_units(prev[0], prev[1]):
                    u()

            if dbg:
                for b in range(B):
                    nc.sync.dma_start(qt2_d[:, b, :], qt2[b][:])
                    nc.sync.dma_start(kt2_d[:, b, :], kt2[b][:])
                    nc.sync.dma_start(v2_d[:, b], v2[b][:])

    nc.compile()
    return nc


def prep_inputs(x, W_q, W_k, W_v, W_o):
    """Build the 8 per-core input maps (numpy, host-side)."""
    bf = ml_dtypes.bfloat16
    # xT: [b, e, l] -> [b, ep(128), et(8), l]
    xt = np.ascontiguousarray(x.transpose(0, 2, 1)).reshape(B, ET, 128, L)
    xt = np.ascontiguousarray(xt.transpose(0, 2, 1, 3)).astype(bf)

    in_maps = []
    for c in range(NCORES):
        h0, h1 = 2 * c, 2 * c + 1
        # [e, 2*64] -> [ep, et, 128]
        def pack(w, scale=1.0):
            m = np.concatenate([w[h0] * scale, w[h1] * scale], axis=1)  # [E, 128]
            m = m.reshape(ET, 128, 128).transpose(1, 0, 2)  # [ep, et, 128]
            return np.ascontiguousarray(m).astype(bf)

        in_maps.append(
            {
                "xt": xt,
                "wq": pack(W_q, 0.125),
                "wk": pack(W_k),
                "wv": pack(W_v),
                "wo": np.ascontiguousarray(W_o[128 * c : 128 * (c + 1), :]).astype(bf),
            }
        )
    return in_maps


def _ensure_ntff_hook():
    """Register the axon NTFF profile hook if the image's antenv lacks it."""
    import types

    try:
        from antenv.axon_hooks import get_axon_ntff_profile_hook  # noqa: F401

        return
    except ImportError:
        pass
    try:
        from trn_agent_boot.trn_boot import _ntff_profile_via_ctypes
    except ImportError:
        return
    so = "/opt/axon/libaxon_pjrt.so"
    if not os.path.exists(so):
        return
    hook = _ntff_profile_via_ctypes(so)
    mod = types.ModuleType("antenv.axon_hooks")
    state = {"hook": hook}
    mod.get_axon_ntff_profile_hook = lambda: state["hook"]
    mod.set_axon_ntff_profile_hook = lambda h: state.update(hook=h)
    import antenv

    antenv.axon_hooks = mod
    sys.modules["antenv.axon_hooks"] = mod


_NC_CACHE = {}


def kernel(x, W_q, W_k, W_v, W_o):
    x = np.asarray(x, dtype=np.float32)
    W_q = np.asarray(W_q, dtype=np.float32)
    W_k = np.asarray(W_k, dtype=np.float32)
    W_v = np.asarray(W_v, dtype=np.float32)
    W_o = np.asarray(W_o, dtype=np.float32)

    if "nc" not in _NC_CACHE:
        _NC_CACHE["nc"] = build_kernel()
    nc = _NC_CACHE["nc"]

    in_maps = prep_inputs(x, W_q, W_k, W_v, W_o)
    if bool(int(os.environ.get("KERNEL_TRACE", "0"))):
        _ensure_ntff_hook()
    res = run_bass_kernel_spmd(
        nc,
        in_maps,
        core_ids=list(range(NCORES)),
        trace=bool(int(os.environ.get("KERNEL_TRACE", "0"))),
    )
    _NC_CACHE["last_results"] = res
    out = np.zeros((B, L, E), dtype=np.float32)
    for r in res.results:
        out += r["out"]
    return out


if __name__ == "__main__":
    # smoke test with random data
    rng = np.random.default_rng(0)
    x = rng.standard_normal((B, L, E), dtype=np.float32)
    wq = (rng.standard_normal((H, E, D)) / np.sqrt(E)).astype(np.float32)
    wk = (rng.standard_normal((H, E, D)) / np.sqrt(E)).astype(np.float32)
    wv = (rng.standard_normal((H, E, D)) / np.sqrt(E)).astype(np.float32)
    wo = (rng.standard_normal((E, E)) / np.sqrt(E)).astype(np.float32)
    out = kernel(x, wq, wk, wv, wo)
    print("out", out.shape, out.dtype, np.abs(out).max())


# revision 48
# speedup vs baseline: 1.0843x; 1.0044x over previous
"""Multi-head self-attention Trainium2 Bass kernel (8 NeuronCores).

Sharding: tensor-parallel over heads. Each core c owns heads {2c, 2c+1}
for all 4 batches:
  - projects q, k, v for its 2 heads (from the full x),
  - computes attention (softmax without max-subtraction: scores are O(+-20),
    safe in fp32),
  - computes the partial output projection with its 128-row slice of W_o.
The 8 partial outputs are summed on the host (the "all-reduce").

Layout strategy (everything transposed so no on-device transposes needed):
  xT   [e, l]    host-prepped
  qT2/kT2 [128=2h*64d, l]  from lhsT=W[e,d2] (2 heads packed), rhs=xT
  v2   [l, 128=2h*64d]     from lhsT=xT, rhs=[Wv_h0|Wv_h1]
  scoresT [lk, lq] = kT.T @ qT  (two heads as concurrent PE row-groups)
  expT = exp(scoresT) on ACT (bf16 out)
  avT' [128=2h*64d, lq]: col-tiled matmuls lhsT=v2-slice, rhs=expT
  denom [1, lq] per head: ones-vector matmuls (col strips 0 / 32)
  concatT = avT * (1/denom broadcast)  -> exactly the lhsT the out-proj needs
  out_partial[l, o] = concatT.T @ W_o_slice
"""

import os
import sys

import numpy as np
import ml_dtypes

import concourse.bass as bass
import concourse.tile as tile
from concourse import bacc, mybir
from concourse.bass_utils import run_bass_kernel_spmd

BF16 = mybir.dt.bfloat16
F32 = mybir.dt.float32
AF = mybir.ActivationFunctionType

B = 4
L = 2048
E = 1024
H = 16
D = 64
NCORES = 8
ET = E // 128  # 8 e-tiles
LT = L // 128  # 16 l-tiles
LQB = 512  # moving free-dim block
NLQ = L // LQB  # 4


def build_kernel(dbg=False):
    nc = bacc.Bacc("TRN2", target_bir_lowering=False, debug=False, num_devices=NCORES)

    xt_d = nc.dram_tensor("xt", [B, 128, ET, L], BF16, kind="ExternalInput")
    wq_d = nc.dram_tensor("wq", [128, ET, 128], BF16, kind="ExternalInput")
    wk_d = nc.dram_tensor("wk", [128, ET, 128], BF16, kind="ExternalInput")
    wv_d = nc.dram_tensor("wv", [128, ET, 128], BF16, kind="ExternalInput")
    wo_d = nc.dram_tensor("wo", [128, E], BF16, kind="ExternalInput")
    out_d = nc.dram_tensor("out", [B, L, E], F32, kind="ExternalOutput")
    if dbg:
        qt2_d = nc.dram_tensor("qt2_dbg", [128, B, L], BF16, kind="ExternalOutput")
        kt2_d = nc.dram_tensor("kt2_dbg", [128, B, L], BF16, kind="ExternalOutput")
        v2_d = nc.dram_tensor("v2_dbg", [128, B, LT, 130], BF16, kind="ExternalOutput")
        rcp_d = nc.dram_tensor("rcp_dbg", [B, NLQ, 2, 1, LQB], F32, kind="ExternalOutput")
        cc_d = nc.dram_tensor("cc_dbg", [B, 2, 64, L], BF16, kind="ExternalOutput")

    with tile.TileContext(nc) as tc:
        with (
            tc.tile_pool(name="persist", bufs=1) as pp,
            tc.tile_pool(name="xin", bufs=3) as xpool,
            tc.tile_pool(name="exp", bufs=3) as epool,
            tc.tile_pool(name="small", bufs=2) as spool,
            tc.tile_pool(name="outp", bufs=3) as opool,
            tc.tile_pool(name="concat", bufs=2) as cpool,
        ):
            # --- persistent SBUF residents ---
            wq_sb = pp.tile([128, ET, 128], BF16, tag="wq")
            wk_sb = pp.tile([128, ET, 128], BF16, tag="wk")
            wv_sb = pp.tile([128, ET, 128], BF16, tag="wv")
            wo_sb = pp.tile([128, E], BF16, tag="wo")
            qt2 = []
            kt2 = []
            v2 = []
            for b in range(B):
                qt2_b = pp.tile([128, L], BF16, tag=f"qt2_{b}")
                kt2_b = pp.tile([128, L], BF16, tag=f"kt2_{b}")
                v2_b = pp.tile([128, LT, 130], BF16, tag=f"v2_{b}")
                qt2.append(qt2_b)
                kt2.append(kt2_b)
                v2.append(v2_b)

            nc.sync.dma_start(wq_sb[:], wq_d[:])
            nc.sync.dma_start(wk_sb[:], wk_d[:])
            nc.sync.dma_start(wv_sb[:], wv_d[:])
            nc.sync.dma_start(wo_sb[:], wo_d[:])
            for b in range(B):
                nc.vector.memset(v2[b][:], 1.0)

            with tc.tile_pool(name="ps", bufs=1, space="PSUM") as ps:

                def proj_chunk_units(b, lc):
                    """Units (callables) projecting q/k/v for l-chunk lc of b."""
                    lsl = bass.ts(lc, LQB)
                    state = {}

                    def u_q():
                        xtile = xpool.tile([128, ET, LQB], BF16, tag="x")
                        nc.sync.dma_start(xtile[:], xt_d[b, :, :, lsl])
                        state["x"] = xtile
                        ps_q = ps.tile([128, LQB], F32, tag="proj", bufs=2)
                        for et in range(ET):
                            nc.tensor.matmul(
                                ps_q[:],
                                wq_sb[:, et, :],
                                xtile[:, et, :],
                                start=(et == 0),
                                stop=(et == ET - 1),
                            )
                        nc.vector.tensor_copy(qt2[b][:, lsl], ps_q[:])

                    def u_k():
                        xtile = state["x"]
                        ps_k = ps.tile([128, LQB], F32, tag="proj", bufs=2)
                        for et in range(ET):
                            nc.tensor.matmul(
                                ps_k[:],
                                wk_sb[:, et, :],
                                xtile[:, et, :],
                                start=(et == 0),
                                stop=(et == ET - 1),
                            )
                        nc.vector.tensor_copy(kt2[b][:, lsl], ps_k[:])

                    def u_v(j):
                        def f():
                            xtile = state["x"]
                            lt = lc * (LQB // 128) + j
                            ps_v = ps.tile([128, 128], F32, tag="proj", bufs=2)
                            for et in range(ET):
                                nc.tensor.matmul(
                                    ps_v[:],
                                    xtile[:, et, bass.ts(j, 128)],
                                    wv_sb[:, et, :],
                                    start=(et == 0),
                                    stop=(et == ET - 1),
                                )
                            nc.vector.tensor_copy(v2[b][:, lt, 0:64], ps_v[:, 0:64])
                            nc.vector.tensor_copy(
                                v2[b][:, lt, 65:129], ps_v[:, 64:128]
                            )

                        return f

                    return [u_q, u_k] + [u_v(j) for j in range(LQB // 128)]

                AV_LAG = 2
                lag_q = []  # pending av-matmul emissions (cross-block lag)

                def flush_av():
                    ps_av, vb, lkt, e2, st, sp, fin = lag_q.pop(0)
                    for h in range(2):
                        nc.tensor.matmul(
                            ps_av[h][:],
                            vb[:, lkt, h * 65 : (h + 1) * 65],
                            e2[:, h, :],
                            start=st,
                            stop=sp,
                        )
                    if fin is not None:
                        fin()

                def normalize_fin(b, lq, cc, ps_av):
                    def fin():
                        lqsl = bass.ts(lq, LQB)
                        for h in range(2):
                            av_sb = spool.tile([65, LQB], F32, tag=f"av_sb{h}")
                            nc.vector.tensor_copy(av_sb[:], ps_av[h][:])
                            dn0 = spool.tile([1, LQB], F32, tag=f"dn0{h}")
                            nc.sync.dma_start(dn0[0:1, :], av_sb[64:65, :])
                            rbd = spool.tile([64, LQB], F32, tag=f"rbd{h}")
                            nc.gpsimd.partition_broadcast(rbd[:], dn0[0:1, :])
                            rbr = spool.tile([64, LQB], F32, tag=f"rbr{h}")
                            nc.vector.reciprocal_approx_fast(out=rbr[:], in_=rbd[:])
                            if h == 0:
                                nc.vector.tensor_mul(
                                    cc[:64, lqsl], av_sb[0:64, :], rbr[:]
                                )
                            else:
                                cctmp = spool.tile([64, LQB], BF16, tag="cctmp")
                                nc.vector.tensor_mul(cctmp[:], av_sb[0:64, :], rbr[:])
                                nc.sync.dma_start(cc[64:128, lqsl], cctmp[:])
                            if dbg:
                                nc.sync.dma_start(rcp_d[b, lq, h], rbr[0:1, :])

                    return fin

                def attention_block(b, lq, cc, bg):
                    """Attention for (b, lq-block): heads as concurrent row-groups.

                    `bg` iterates background-work callables (one per lkt) to
                    fill PE slack. av matmuls are emitted through lag_q with a
                    cross-block lag so no PE matmul ever waits on a just-issued
                    exp."""
                    lqsl = bass.ts(lq, LQB)
                    ps_av0 = ps.tile([65, LQB], F32, tag="av0", bufs=1)
                    ps_av1 = ps.tile([65, LQB], F32, tag="av1", bufs=1)
                    ps_av = [ps_av0, ps_av1]
                    fin = normalize_fin(b, lq, cc, ps_av)
                    for lkt in range(LT):
                        ps_s = ps.tile([128, 2, LQB], F32, tag="s", bufs=2)
                        for h in range(2):
                            hsl = slice(h * 64, (h + 1) * 64)
                            nc.tensor.matmul(
                                ps_s[:, h, :],
                                kt2[b][hsl, bass.ts(lkt, 128)],
                                qt2[b][hsl, lqsl],
                                start=True,
                                stop=True,
                            )
                        e2 = epool.tile([128, 2, LQB], BF16, tag="e", bufs=6)
                        nc.scalar.activation(e2[:], ps_s[:], AF.Exp)
                        lag_q.append(
                            (
                                ps_av,
                                v2[b],
                                lkt,
                                e2,
                                lkt == 0,
                                lkt == LT - 1,
                                fin if lkt == LT - 1 else None,
                            )
                        )
                        if len(lag_q) > AV_LAG:
                            flush_av()
                        u = next(bg, None)
                        if u is not None:
                            u()

                def outproj_units(b, cc):
                    def u(lt, oc):
                        def f():
                            ps_o = ps.tile([128, 512], F32, tag="proj", bufs=2)
                            nc.tensor.matmul(
                                ps_o[:],
                                cc[:, bass.ts(lt, 128)],
                                wo_sb[:, bass.ts(oc, 512)],
                                start=True,
                                stop=True,
                            )
                            out_t = opool.tile([128, 512], F32, tag="out")
                            nc.vector.tensor_copy(out_t[:], ps_o[:])
                            nc.sync.dma_start(
                                out_d[b, bass.ts(lt, 128), bass.ts(oc, 512)], out_t[:]
                            )

                        return f

                    return [u(lt, oc) for lt in range(LT) for oc in range(2)]

                # prologue: project batch 0; spread proj(b+1) and
                # outproj(b-1) units under attention(b)'s lkt loop
                for lc in range(NLQ):
                    for u in proj_chunk_units(0, lc):
                        u()
                prev = None  # (b, cc) awaiting out-projection
                for b in range(B):
                    cc = cpool.tile([128, L], BF16, tag="cc")
                    units = []
                    if b + 1 < B:
                        for lc in range(NLQ):
                            units += proj_chunk_units(b + 1, lc)
                    if prev is not None:
                        units += outproj_units(prev[0], prev[1])
                    bg = iter(units)
                    for lq in range(NLQ):
                        attention_block(b, lq, cc, bg)
                    for u in bg:
                        u()
                    if dbg:
                        nc.sync.dma_start(cc_d[b, 0], cc[0:64, :])
                        nc.sync.dma_start(cc_d[b, 1], cc[64:128, :])
                    prev = (b, cc)
                while lag_q:
                    flush_av()
                for u in outproj_units(prev[0], prev[1]):
                    u()

            if dbg:
                for b in range(B):
                    nc.sync.dma_start(qt2_d[:, b, :], qt2[b][:])
                    nc.sync.dma_start(kt2_d[:, b, :], kt2[b][:])
                    nc.sync.dma_start(v2_d[:, b], v2[b][:])

    nc.compile()
    return nc


def prep_inputs(x, W_q, W_k, W_v, W_o):
    """Build the 8 per-core input maps (numpy, host-side)."""
    bf = ml_dtypes.bfloat16
    # xT: [b, e, l] -> [b, ep(128), et(8), l]
    xt = np.ascontiguousarray(x.transpose(0, 2, 1)).reshape(B, ET, 128, L)
    xt = np.ascontiguousarray(xt.transpose(0, 2, 1, 3)).astype(bf)

    in_maps = []
    for c in range(NCORES):
        h0, h1 = 2 * c, 2 * c + 1
        # [e, 2*64] -> [ep, et, 128]
        def pack(w, scale=1.0):
            m = np.concatenate([w[h0] * scale, w[h1] * scale], axis=1)  # [E, 128]
            m = m.reshape(ET, 128, 128).transpose(1, 0, 2)  # [ep, et, 128]
            return np.ascontiguousarray(m).astype(bf)

        in_maps.append(
            {
                "xt": xt,
                "wq": pack(W_q, 0.125),
                "wk": pack(W_k),
                "wv": pack(W_v),
                "wo": np.ascontiguousarray(W_o[128 * c : 128 * (c + 1), :]).astype(bf),
            }
        )
    return in_maps


def _ensure_ntff_hook():
    """Register the axon NTFF profile hook if the image's antenv lacks it."""
    import types

    try:
        from antenv.axon_hooks import get_axon_ntff_profile_hook  # noqa: F401

        return
    except ImportError:
        pass
    try:
        from trn_agent_boot.trn_boot import _ntff_profile_via_ctypes
    except ImportError:
        return
    so = "/opt/axon/libaxon_pjrt.so"
    if not os.path.exists(so):
        return
    hook = _ntff_profile_via_ctypes(so)
    mod = types.ModuleType("antenv.axon_hooks")
    state = {"hook": hook}
    mod.get_axon_ntff_profile_hook = lambda: state["hook"]
    mod.set_axon_ntff_profile_hook = lambda h: state.update(hook=h)
    import antenv

    antenv.axon_hooks = mod
    sys.modules["antenv.axon_hooks"] = mod


_NC_CACHE = {}


def kernel(x, W_q, W_k, W_v, W_o):
    x = np.asarray(x, dtype=np.float32)
    W_q = np.asarray(W_q, dtype=np.float32)
    W_k = np.asarray(W_k, dtype=np.float32)
    W_v = np.asarray(W_v, dtype=np.float32)
    W_o = np.asarray(W_o, dtype=np.float32)

    if "nc" not in _NC_CACHE:
        _NC_CACHE["nc"] = build_kernel()
    nc = _NC_CACHE["nc"]

    in_maps = prep_inputs(x, W_q, W_k, W_v, W_o)
    if bool(int(os.environ.get("KERNEL_TRACE", "0"))):
        _ensure_ntff_hook()
    res = run_bass_kernel_spmd(
        nc,
        in_maps,
        core_ids=list(range(NCORES)),
        trace=bool(int(os.environ.get("KERNEL_TRACE", "0"))),
    )
    _NC_CACHE["last_results"] = res
    out = np.zeros((B, L, E), dtype=np.float32)
    for r in res.results:
        out += r["out"]
    return out


if __name__ == "__main__":
    # smoke test with random data
    rng = np.random.default_rng(0)
    x = rng.standard_normal((B, L, E), dtype=np.float32)
    wq = (rng.standard_normal((H, E, D)) / np.sqrt(E)).astype(np.float32)
    wk = (rng.standard_normal((H, E, D)) / np.sqrt(E)).astype(np.float32)
    wv = (rng.standard_normal((H, E, D)) / np.sqrt(E)).astype(np.float32)
    wo = (rng.standard_normal((E, E)) / np.sqrt(E)).astype(np.float32)
    out = kernel(x, wq, wk, wv, wo)
    print("out", out.shape, out.dtype, np.abs(out).max())


# revision 50
# speedup vs baseline: 1.0887x; 1.0041x over previous
"""Multi-head self-attention Trainium2 Bass kernel (8 NeuronCores).

Sharding: tensor-parallel over heads. Each core c owns heads {2c, 2c+1}
for all 4 batches:
  - projects q, k, v for its 2 heads (from the full x),
  - computes attention (softmax without max-subtraction: scores are O(+-20),
    safe in fp32),
  - computes the partial output projection with its 128-row slice of W_o.
The 8 partial outputs are summed on the host (the "all-reduce").

Layout strategy (everything transposed so no on-device transposes needed):
  xT   [e, l]    host-prepped
  qT2/kT2 [128=2h*64d, l]  from lhsT=W[e,d2] (2 heads packed), rhs=xT
  v2   [l, 128=2h*64d]     from lhsT=xT, rhs=[Wv_h0|Wv_h1]
  scoresT [lk, lq] = kT.T @ qT  (two heads as concurrent PE row-groups)
  expT = exp(scoresT) on ACT (bf16 out)
  avT' [128=2h*64d, lq]: col-tiled matmuls lhsT=v2-slice, rhs=expT
  denom [1, lq] per head: ones-vector matmuls (col strips 0 / 32)
  concatT = avT * (1/denom broadcast)  -> exactly the lhsT the out-proj needs
  out_partial[l, o] = concatT.T @ W_o_slice
"""

import os
import sys

import numpy as np
import ml_dtypes

import concourse.bass as bass
import concourse.tile as tile
from concourse import bacc, mybir
from concourse.bass_utils import run_bass_kernel_spmd

BF16 = mybir.dt.bfloat16
F32 = mybir.dt.float32
AF = mybir.ActivationFunctionType

B = 4
L = 2048
E = 1024
H = 16
D = 64
NCORES = 8
ET = E // 128  # 8 e-tiles
LT = L // 128  # 16 l-tiles
LQB = 512  # moving free-dim block
NLQ = L // LQB  # 4


def build_kernel(dbg=False):
    nc = bacc.Bacc("TRN2", target_bir_lowering=False, debug=False, num_devices=NCORES)

    xt_d = nc.dram_tensor("xt", [B, 128, ET, L], BF16, kind="ExternalInput")
    wq_d = nc.dram_tensor("wq", [128, ET, 128], BF16, kind="ExternalInput")
    wk_d = nc.dram_tensor("wk", [128, ET, 128], BF16, kind="ExternalInput")
    wv_d = nc.dram_tensor("wv", [128, ET, 128], BF16, kind="ExternalInput")
    wo_d = nc.dram_tensor("wo", [128, E], BF16, kind="ExternalInput")
    out_d = nc.dram_tensor("out", [B, L, E], F32, kind="ExternalOutput")
    if dbg:
        qt2_d = nc.dram_tensor("qt2_dbg", [128, B, L], BF16, kind="ExternalOutput")
        kt2_d = nc.dram_tensor("kt2_dbg", [128, B, L], BF16, kind="ExternalOutput")
        v2_d = nc.dram_tensor("v2_dbg", [128, B, LT, 130], BF16, kind="ExternalOutput")
        rcp_d = nc.dram_tensor("rcp_dbg", [B, NLQ, 2, 1, LQB], F32, kind="ExternalOutput")
        cc_d = nc.dram_tensor("cc_dbg", [B, 2, 64, L], BF16, kind="ExternalOutput")

    with tile.TileContext(nc) as tc:
        with (
            tc.tile_pool(name="persist", bufs=1) as pp,
            tc.tile_pool(name="xin", bufs=3) as xpool,
            tc.tile_pool(name="exp", bufs=3) as epool,
            tc.tile_pool(name="small", bufs=2) as spool,
            tc.tile_pool(name="outp", bufs=3) as opool,
            tc.tile_pool(name="concat", bufs=2) as cpool,
        ):
            # --- persistent SBUF residents ---
            wq_sb = pp.tile([128, ET, 128], BF16, tag="wq")
            wk_sb = pp.tile([128, ET, 128], BF16, tag="wk")
            wv_sb = pp.tile([128, ET, 128], BF16, tag="wv")
            wo_sb = pp.tile([128, E], BF16, tag="wo")
            qt2 = []
            kt2 = []
            v2 = []
            for b in range(B):
                qt2_b = pp.tile([128, L], BF16, tag=f"qt2_{b}")
                kt2_b = pp.tile([128, L], BF16, tag=f"kt2_{b}")
                v2_b = pp.tile([128, LT, 130], BF16, tag=f"v2_{b}")
                qt2.append(qt2_b)
                kt2.append(kt2_b)
                v2.append(v2_b)

            nc.sync.dma_start(wq_sb[:], wq_d[:])
            nc.sync.dma_start(wk_sb[:], wk_d[:])
            nc.sync.dma_start(wv_sb[:], wv_d[:])
            nc.sync.dma_start(wo_sb[:], wo_d[:])
            for b in range(B):
                nc.vector.memset(v2[b][:], 1.0)

            with tc.tile_pool(name="ps", bufs=1, space="PSUM") as ps:

                def proj_chunk_units(b, lc):
                    """Units (callables) projecting q/k/v for l-chunk lc of b."""
                    lsl = bass.ts(lc, LQB)
                    state = {}

                    def u_q0():
                        xtile = xpool.tile([128, ET, LQB], BF16, tag="x")
                        nc.sync.dma_start(xtile[:], xt_d[b, :, :, lsl])
                        state["x"] = xtile
                        ps_q = ps.tile([128, LQB], F32, tag="proj", bufs=2)
                        state["q"] = ps_q
                        for et in range(ET // 2):
                            nc.tensor.matmul(
                                ps_q[:],
                                wq_sb[:, et, :],
                                xtile[:, et, :],
                                start=(et == 0),
                                stop=False,
                            )

                    def u_q1():
                        xtile, ps_q = state["x"], state["q"]
                        for et in range(ET // 2, ET):
                            nc.tensor.matmul(
                                ps_q[:],
                                wq_sb[:, et, :],
                                xtile[:, et, :],
                                start=False,
                                stop=(et == ET - 1),
                            )
                        nc.vector.tensor_copy(qt2[b][:, lsl], ps_q[:])

                    def u_k0():
                        xtile = state["x"]
                        ps_k = ps.tile([128, LQB], F32, tag="proj", bufs=2)
                        state["k"] = ps_k
                        for et in range(ET // 2):
                            nc.tensor.matmul(
                                ps_k[:],
                                wk_sb[:, et, :],
                                xtile[:, et, :],
                                start=(et == 0),
                                stop=False,
                            )

                    def u_k1():
                        xtile, ps_k = state["x"], state["k"]
                        for et in range(ET // 2, ET):
                            nc.tensor.matmul(
                                ps_k[:],
                                wk_sb[:, et, :],
                                xtile[:, et, :],
                                start=False,
                                stop=(et == ET - 1),
                            )
                        nc.vector.tensor_copy(kt2[b][:, lsl], ps_k[:])

                    def u_v(j):
                        def f():
                            xtile = state["x"]
                            lt = lc * (LQB // 128) + j
                            ps_v = ps.tile([128, 128], F32, tag="proj", bufs=2)
                            for et in range(ET):
                                nc.tensor.matmul(
                                    ps_v[:],
                                    xtile[:, et, bass.ts(j, 128)],
                                    wv_sb[:, et, :],
                                    start=(et == 0),
                                    stop=(et == ET - 1),
                                )
                            nc.vector.tensor_copy(v2[b][:, lt, 0:64], ps_v[:, 0:64])
                            nc.vector.tensor_copy(
                                v2[b][:, lt, 65:129], ps_v[:, 64:128]
                            )

                        return f

                    return [u_q0, u_q1, u_k0, u_k1] + [
                        u_v(j) for j in range(LQB // 128)
                    ]

                AV_LAG = 2
                lag_q = []  # pending av-matmul emissions (cross-block lag)

                def flush_av():
                    ps_av, vb, lkt, e2, st, sp, fin = lag_q.pop(0)
                    for h in range(2):
                        nc.tensor.matmul(
                            ps_av[h][:],
                            vb[:, lkt, h * 65 : (h + 1) * 65],
                            e2[:, h, :],
                            start=st,
                            stop=sp,
                        )
                    if fin is not None:
                        fin()

                def normalize_fin(b, lq, cc, ps_av):
                    def fin():
                        lqsl = bass.ts(lq, LQB)
                        for h in range(2):
                            av_sb = spool.tile([65, LQB], F32, tag=f"av_sb{h}")
                            nc.vector.tensor_copy(av_sb[:], ps_av[h][:])
                            dn0 = spool.tile([1, LQB], F32, tag=f"dn0{h}")
                            nc.sync.dma_start(dn0[0:1, :], av_sb[64:65, :])
                            rbd = spool.tile([64, LQB], F32, tag=f"rbd{h}")
                            nc.gpsimd.partition_broadcast(rbd[:], dn0[0:1, :])
                            rbr = spool.tile([64, LQB], F32, tag=f"rbr{h}")
                            nc.vector.reciprocal_approx_fast(out=rbr[:], in_=rbd[:])
                            if h == 0:
                                nc.vector.tensor_mul(
                                    cc[:64, lqsl], av_sb[0:64, :], rbr[:]
                                )
                            else:
                                cctmp = spool.tile([64, LQB], BF16, tag="cctmp")
                                nc.vector.tensor_mul(cctmp[:], av_sb[0:64, :], rbr[:])
                                nc.sync.dma_start(cc[64:128, lqsl], cctmp[:])
                            if dbg:
                                nc.sync.dma_start(rcp_d[b, lq, h], rbr[0:1, :])

                    return fin

                def attention_block(b, lq, cc, bg):
                    """Attention for (b, lq-block): heads as concurrent row-groups.

                    `bg` iterates background-work callables (one per lkt) to
                    fill PE slack. av matmuls are emitted through lag_q with a
                    cross-block lag so no PE matmul ever waits on a just-issued
                    exp."""
                    lqsl = bass.ts(lq, LQB)
                    ps_av0 = ps.tile([65, LQB], F32, tag="av0", bufs=1)
                    ps_av1 = ps.tile([65, LQB], F32, tag="av1", bufs=1)
                    ps_av = [ps_av0, ps_av1]
                    fin = normalize_fin(b, lq, cc, ps_av)
                    for lkt in range(LT):
                        ps_s = ps.tile([128, 2, LQB], F32, tag="s", bufs=2)
                        for h in range(2):
                            hsl = slice(h * 64, (h + 1) * 64)
                            nc.tensor.matmul(
                                ps_s[:, h, :],
                                kt2[b][hsl, bass.ts(lkt, 128)],
                                qt2[b][hsl, lqsl],
                                start=True,
                                stop=True,
                            )
                        e2 = epool.tile([128, 2, LQB], BF16, tag="e", bufs=6)
                        nc.scalar.activation(e2[:], ps_s[:], AF.Exp)
                        lag_q.append(
                            (
                                ps_av,
                                v2[b],
                                lkt,
                                e2,
                                lkt == 0,
                                lkt == LT - 1,
                                fin if lkt == LT - 1 else None,
                            )
                        )
                        if len(lag_q) > AV_LAG:
                            flush_av()
                        u = next(bg, None)
                        if u is not None:
                            u()

                def outproj_units(b, cc):
                    def u(lt, oc):
                        def f():
                            ps_o = ps.tile([128, 512], F32, tag="proj", bufs=2)
                            nc.tensor.matmul(
                                ps_o[:],
                                cc[:, bass.ts(lt, 128)],
                                wo_sb[:, bass.ts(oc, 512)],
                                start=True,
                                stop=True,
                            )
                            out_t = opool.tile([128, 512], F32, tag="out")
                            nc.vector.tensor_copy(out_t[:], ps_o[:])
                            nc.sync.dma_start(
                                out_d[b, bass.ts(lt, 128), bass.ts(oc, 512)], out_t[:]
                            )

                        return f

                    return [u(lt, oc) for lt in range(LT) for oc in range(2)]

                # prologue: project batch 0; spread proj(b+1) and
                # outproj(b-1) units under attention(b)'s lkt loop
                for lc in range(NLQ):
                    for u in proj_chunk_units(0, lc):
                        u()
                prev = None  # (b, cc) awaiting out-projection
                for b in range(B):
                    cc = cpool.tile([128, L], BF16, tag="cc")
                    units = []
                    if b + 1 < B:
                        for lc in range(NLQ):
                            units += proj_chunk_units(b + 1, lc)
                    if prev is not None:
                        units += outproj_units(prev[0], prev[1])
                    bg = iter(units)
                    for lq in range(NLQ):
                        attention_block(b, lq, cc, bg)
                    for u in bg:
                        u()
                    if dbg:
                        nc.sync.dma_start(cc_d[b, 0], cc[0:64, :])
                        nc.sync.dma_start(cc_d[b, 1], cc[64:128, :])
                    prev = (b, cc)
                while lag_q:
                    flush_av()
                for u in outproj_units(prev[0], prev[1]):
                    u()

            if dbg:
                for b in range(B):
                    nc.sync.dma_start(qt2_d[:, b, :], qt2[b][:])
                    nc.sync.dma_start(kt2_d[:, b, :], kt2[b][:])
                    nc.sync.dma_start(v2_d[:, b], v2[b][:])

    nc.compile()
    return nc


def prep_inputs(x, W_q, W_k, W_v, W_o):
    """Build the 8 per-core input maps (numpy, host-side)."""
    bf = ml_dtypes.bfloat16
    # xT: [b, e, l] -> [b, ep(128), et(8), l]
    xt = np.ascontiguousarray(x.transpose(0, 2, 1)).reshape(B, ET, 128, L)
    xt = np.ascontiguousarray(xt.transpose(0, 2, 1, 3)).astype(bf)

    in_maps = []
    for c in range(NCORES):
        h0, h1 = 2 * c, 2 * c + 1
        # [e, 2*64] -> [ep, et, 128]
        def pack(w, scale=1.0):
            m = np.concatenate([w[h0] * scale, w[h1] * scale], axis=1)  # [E, 128]
            m = m.reshape(ET, 128, 128).transpose(1, 0, 2)  # [ep, et, 128]
            return np.ascontiguousarray(m).astype(bf)

        in_maps.append(
            {
                "xt": xt,
                "wq": pack(W_q, 0.125),
                "wk": pack(W_k),
                "wv": pack(W_v),
                "wo": np.ascontiguousarray(W_o[128 * c : 128 * (c + 1), :]).astype(bf),
            }
        )
    return in_maps


def _ensure_ntff_hook():
    """Register the axon NTFF profile hook if the image's antenv lacks it."""
    import types

    try:
        from antenv.axon_hooks import get_axon_ntff_profile_hook  # noqa: F401

        return
    except ImportError:
        pass
    try:
        from trn_agent_boot.trn_boot import _ntff_profile_via_ctypes
    except ImportError:
        return
    so = "/opt/axon/libaxon_pjrt.so"
    if not os.path.exists(so):
        return
    hook = _ntff_profile_via_ctypes(so)
    mod = types.ModuleType("antenv.axon_hooks")
    state = {"hook": hook}
    mod.get_axon_ntff_profile_hook = lambda: state["hook"]
    mod.set_axon_ntff_profile_hook = lambda h: state.update(hook=h)
    import antenv

    antenv.axon_hooks = mod
    sys.modules["antenv.axon_hooks"] = mod


_NC_CACHE = {}


def kernel(x, W_q, W_k, W_v, W_o):
    x = np.asarray(x, dtype=np.float32)
    W_q = np.asarray(W_q, dtype=np.float32)
    W_k = np.asarray(W_k, dtype=np.float32)
    W_v = np.asarray(W_v, dtype=np.float32)
    W_o = np.asarray(W_o, dtype=np.float32)

    if "nc" not in _NC_CACHE:
        _NC_CACHE["nc"] = build_kernel()
    nc = _NC_CACHE["nc"]

    in_maps = prep_inputs(x, W_q, W_k, W_v, W_o)
    if bool(int(os.environ.get("KERNEL_TRACE", "0"))):
        _ensure_ntff_hook()
    res = run_bass_kernel_spmd(
        nc,
        in_maps,
        core_ids=list(range(NCORES)),
        trace=bool(int(os.environ.get("KERNEL_TRACE", "0"))),
    )
    _NC_CACHE["last_results"] = res
    out = np.zeros((B, L, E), dtype=np.float32)
    for r in res.results:
        out += r["out"]
    return out


if __name__ == "__main__":
    # smoke test with random data
    rng = np.random.default_rng(0)
    x = rng.standard_normal((B, L, E), dtype=np.float32)
    wq = (rng.standard_normal((H, E, D)) / np.sqrt(E)).astype(np.float32)
    wk = (rng.standard_normal((H, E, D)) / np.sqrt(E)).astype(np.float32)
    wv = (rng.standard_normal((H, E, D)) / np.sqrt(E)).astype(np.float32)
    wo = (rng.standard_normal((E, E)) / np.sqrt(E)).astype(np.float32)
    out = kernel(x, wq, wk, wv, wo)
    print("out", out.shape, out.dtype, np.abs(out).max())
